# revision 1
# baseline (speedup 1.0000x reference)
"""GAT layer (PyG GATConv semantics) on 8 Trainium2 NeuronCores.

Strategy (edge/graph parallel, dst-sharded):
  - Append self-loops; partition destination nodes into 784 windows of 128.
  - Rank windows by edge count; window rank-group g supplies slot g of each
    of the 8 cores, so all cores share one compile-time schedule.
  - Each core: builds the full node table T1[n] = [h(n) | a_src(n)] (bf16,
    512B rows) from x @ [W | W@A] on the TensorEngine, plus a local
    per-dst-shard table T2 = [a_dst | pad] (bf16, 256B rows).
  - Edge phase: dma_gather rows of T1 by edge src (4 table chunks, int16
    local indices) and rows of T2 by edge dst; per 128-edge tile build a
    one-hot selection matrix Sel[e,d] = (dstloc_e == d) and accumulate
      psum[d, 0:128] += Sel.T @ (w ⊙ h_src);  psum[d, 128:132] += Sel.T @ w
    where w = exp(leaky_relu(a_src[src] + a_dst[dst])) = max(exp(z),
    exp(0.2 z)).  Finally out[d] = psum[d]/s[d] + bias, written per window.
"""

import math
import os

import numpy as np
import ml_dtypes

import concourse.bacc as bacc
import concourse.bass as bass
import concourse.mybir as mybir
import concourse.tile as tile
from concourse.library_config import mlp
from concourse.bass_utils import run_bass_kernel_spmd
from concourse.masks import make_identity
from concourse.vector_clock import ScopedClock

BF16 = ml_dtypes.bfloat16

N = 100000
E = 1600000
IN_DIM = 128
HEADS = 4
CDIM = 32
NCORES = 8
P = 128

NP_ = 100352            # N padded to 784 x-tiles of 128
NWIN = NP_ // P         # 784 global windows
SLOTS = NWIN // NCORES  # 98 slots per core
CHUNK = NP_ // 4        # 25088 rows per T1 chunk (int16-indexable)
SHARD = SLOTS * P       # 12544 dst nodes per core
PADROW = SHARD          # T2 pad row (a_dst = -60 => w ~ 0)
SUPB = 8                # slots per superblock (gather batching)

_NEG = -60.0


# ---------------------------------------------------------------------------
# walrus workaround: this container's walrus accepts ONE sem wait per
# instruction; TileContext's tail drain accumulates many. Split extras onto
# single-wait EventSemaphore instructions.
def _split_multi_waits(nc):
    n = [0]

    def fresh():
        n[0] += 1
        return f"waitsplit-{n[0]}"

    for fn in nc.m.functions:
        for bb in fn.blocks:
            insts = list(bb.instructions)
            if not any(
                i.sync_info is not None and len(i.sync_info.on_wait) > 1
                for i in insts
            ):
                continue
            out = []
            for inst in insts:
                si = inst.sync_info
                if si is not None and len(si.on_wait) > 1:
                    waits = list(si.on_wait)
                    for w in waits[:-1]:
                        out.append(mybir.InstEventSemaphore(
                            name=fresh(), opcode="EventSemaphore",
                            engine=inst.engine,
                            sync_info=mybir.SyncInfo(on_wait=[w], on_update=[]),
                        ))
                    si.on_wait = waits[-1:]
                out.append(inst)
            bb.instructions = out


def _wrap_idx(seg):
    """dma_gather index layout: wrap in 16 partitions, replicate x8."""
    assert seg.size % 128 == 0
    return np.tile(seg.reshape(-1, 16).T, (8, 1)).astype(np.int16)


# ---------------------------------------------------------------------------
def _host_prep(x, edge_index):
    """Build the per-core schedule + data arrays. Pure indexing, no FP math."""
    src = np.concatenate([edge_index[0].astype(np.int64), np.arange(N)])
    dst = np.concatenate([edge_index[1].astype(np.int64), np.arange(N)])
    win = dst >> 7

    wcount = np.bincount(win, minlength=NWIN)
    order = np.argsort(-wcount, kind="stable")        # windows by size desc
    core_of_win = np.empty(NWIN, np.int64)
    slot_of_win = np.empty(NWIN, np.int64)
    core_of_win[order] = np.arange(NWIN) % NCORES
    slot_of_win[order] = np.arange(NWIN) // NCORES

    chunk = src // CHUNK
    wc = np.bincount(win * 4 + chunk, minlength=NWIN * 4).reshape(NWIN, 4)
    # caps[g][c]: tiles for chunk-c segment of slot g (max over the 8 cores)
    grp = order.reshape(SLOTS, NCORES)
    caps = np.ceil(wc[grp].max(axis=1) / P).astype(np.int64)   # [SLOTS, 4]
    caps = np.maximum(caps, 0)

    # stream layout: position ordered by (supb, chunk, slot, tile, lane)
    supb_sizes = [SUPB] * (SLOTS // SUPB) + ([SLOTS % SUPB] if SLOTS % SUPB else [])
    seg_tiles = []          # (s, c) -> tiles
    slot_seg_off = np.zeros((SLOTS, 4), np.int64)   # tile offset of (g, c) run
    tcursor = 0
    sb0 = 0
    for sb, nsl in enumerate(supb_sizes):
        for c in range(4):
            for j in range(nsl):
                g = sb0 + j
                slot_seg_off[g, c] = tcursor
                tcursor += caps[g, c]
            seg_tiles.append((sb, c, int(caps[sb0:sb0 + nsl, c].sum())))
        sb0 += nsl
    T_tot = tcursor

    # per-core arrays
    ecore = core_of_win[win]
    eslot = slot_of_win[win]
    cores = []
    for k in range(NCORES):
        m = np.nonzero(ecore == k)[0]
        es, ed, ec, eg = src[m], dst[m], chunk[m], eslot[m]
        o = np.lexsort((ed, ec, eg))
        es, ed, ec, eg = es[o], ed[o], ec[o], eg[o]
        # rank within (slot, chunk) group
        key = eg * 4 + ec
        start = np.searchsorted(key, np.arange(SLOTS * 4))
        rank = np.arange(len(es)) - start[key]
        pos = slot_seg_off[eg, ec] * P + rank
        g1 = np.zeros(T_tot * P, np.int16)                      # pad: row 0
        g2 = np.full(T_tot * P, PADROW, np.int16)               # pad: -60 row
        dl = np.zeros(T_tot * P, np.int16)                      # pad: d 0
        g1[pos] = (es - ec * CHUNK).astype(np.int16)
        g2[pos] = (eg * P + (ed & 127)).astype(np.int16)
        dl[pos] = (ed & 127).astype(np.int16)
        cores.append({"g1": g1, "g2": g2, "dl": dl})

    sched = {
        "caps": caps, "supb_sizes": supb_sizes, "seg_tiles": seg_tiles,
        "T_tot": T_tot, "order": order, "grp": grp,
        "core_of_win": core_of_win, "slot_of_win": slot_of_win,
    }
    return cores, sched


def _pack_core_arrays(core, sched):
    """Wrap index streams per gather instruction; dstloc per tile column."""
    T_tot = sched["T_tot"]
    g1_parts, g2_parts = [], []
    t0 = 0
    for (sb, c, tiles) in sched["seg_tiles"]:
        seg = core["g1"][t0 * P:(t0 + tiles) * P]
        if tiles:
            g1_parts.append(_wrap_idx(seg))
        t0 += tiles
    # g2: same per-(supb, chunk) segmentation as g1 (descriptor-ring cap)
    t0 = 0
    for (sb, c, tiles) in sched["seg_tiles"]:
        seg = core["g2"][t0 * P:(t0 + tiles) * P]
        if tiles:
            g2_parts.append(_wrap_idx(seg))
        t0 += tiles
    g1w = np.concatenate(g1_parts, axis=1) if g1_parts else np.zeros((128, 0), np.int16)
    g2w = np.concatenate(g2_parts, axis=1) if g2_parts else np.zeros((128, 0), np.int16)
    dl = core["dl"].reshape(T_tot, P).T.astype(BF16).copy()
    return g1w, g2w, dl


# ---------------------------------------------------------------------------
def _build_nc(sched):
    caps = sched["caps"]
    supb_sizes = sched["supb_sizes"]
    T_tot = sched["T_tot"]
    AF = mybir.ActivationFunctionType
    AL = mybir.AluOpType
    f32, bf16, i16 = mybir.dt.float32, mybir.dt.bfloat16, mybir.dt.int16

    g1cols = sum(t * 8 for (_, _, t) in sched["seg_tiles"])
    g2cols = T_tot * 8

    nc = bacc.Bacc("TRN2")
    xT = nc.declare_dram_parameter("xT", [P, NP_], f32, isOutput=False)
    xsT = nc.declare_dram_parameter("xsT", [P, SHARD], f32, isOutput=False)
    Wp = nc.declare_dram_parameter("W", [P, P], f32, isOutput=False)
    Acat = nc.declare_dram_parameter("Acat", [P, 8], f32, isOutput=False)
    biasr = nc.declare_dram_parameter("biasr", [P, P], f32, isOutput=False)
    iotap = nc.declare_dram_parameter("iota", [P, P], bf16, isOutput=False)
    negrow = nc.declare_dram_parameter("negrow", [1, P], bf16, isOutput=False)
    g1i = nc.declare_dram_parameter("g1i", [P, max(g1cols, 8)], i16, isOutput=False)
    g2i = nc.declare_dram_parameter("g2i", [P, max(g2cols, 8)], i16, isOutput=False)
    dlp = nc.declare_dram_parameter("dloc", [P, max(T_tot, 1)], bf16, isOutput=False)
    outp = nc.declare_dram_parameter("out", [SHARD, P], f32, isOutput=True)

    T1 = nc.dram_tensor("t1", [NP_, 256], bf16)
    T2 = nc.dram_tensor("t2", [SHARD + 1, P], bf16)

    nc.gpsimd.load_library(mlp)

    with tile.TileContext(nc) as tc:
        with tc.tile_pool(name="const", bufs=1) as cpool:
            ident = cpool.tile([P, P], f32)
            make_identity(nc, ident[:])
            iot = cpool.tile([P, P], bf16)
            nc.sync.dma_start(out=iot[:], in_=iotap[:])
            bias_t = cpool.tile([P, P], f32)
            nc.sync.dma_start(out=bias_t[:], in_=biasr[:])
            wprime = cpool.tile([P, 136], f32)
            nc.sync.dma_start(out=wprime[:, 0:128], in_=Wp[:])
            acat_t = cpool.tile([P, 8], f32)
            nc.sync.dma_start(out=acat_t[:], in_=Acat[:])
            # ---------------- table build ----------------
            with tc.tile_pool(name="tb", bufs=3) as tb, \
                 tc.tile_pool(name="tbp", bufs=2, space="PSUM") as tbp:
                # W' cols 128:136 = W @ Acat  (contract over out-features)
                wtp = tbp.tile([P, P], f32, space="PSUM")
                nc.tensor.transpose(out=wtp[:], in_=wprime[:, 0:128], identity=ident[:])
                wT = tb.tile([P, P], f32)
                nc.vector.tensor_copy(out=wT[:], in_=wtp[:])
                wap = tbp.tile([P, 8], f32, space="PSUM")
                nc.tensor.matmul(out=wap[:], lhsT=wT[:], rhs=acat_t[:],
                                 start=True, stop=True)
                nc.vector.tensor_copy(out=wprime[:, 128:136], in_=wap[:])
                NBLK = 8
                for b in range(NP_ // P // NBLK):
                    xt = tb.tile([P, NBLK * P], f32, tag="xt")
                    nc.sync.dma_start(
                        out=xt[:], in_=xT[:, b * NBLK * P:(b + 1) * NBLK * P])
                    st = tb.tile([P, NBLK * 256], bf16, tag="st")
                    nc.gpsimd.memset(st[:], 0)
                    for t in range(NBLK):
                        ps = tbp.tile([P, 136], f32, space="PSUM", tag="ps")
                        nc.tensor.matmul(
                            out=ps[:], lhsT=xt[:, t * P:(t + 1) * P],
                            rhs=wprime[:], start=True, stop=True)
                        if t % 2 == 0:
                            nc.vector.tensor_copy(
                                out=st[:, t * 256:t * 256 + 136], in_=ps[:])
                        else:
                            nc.scalar.activation(
                                out=st[:, t * 256:t * 256 + 136], in_=ps[:],
                                func=AF.Copy)
                    nc.sync.dma_start(
                        out=T1[b * NBLK * P:(b + 1) * NBLK * P, :].rearrange(
                            "(t p) c -> p t c", p=P),
                        in_=st[:].rearrange("p (t c) -> p t c", t=NBLK))
                for j in range(SLOTS):
                    xt = tb.tile([P, P], f32, tag="xt")
                    nc.sync.dma_start(out=xt[:], in_=xsT[:, j * P:(j + 1) * P])
                    ps = tbp.tile([P, 136], f32, space="PSUM", tag="ps")
                    nc.tensor.matmul(out=ps[:], lhsT=xt[:], rhs=wprime[:],
                                     start=True, stop=True)
                    st2 = tb.tile([P, P], bf16, tag="st2")
                    nc.gpsimd.memset(st2[:, 4:128], 0)
                    nc.vector.tensor_copy(out=st2[:, 0:4], in_=ps[:, 132:136])
                    nc.sync.dma_start(out=T2[j * P:(j + 1) * P, :], in_=st2[:])
                ng = tb.tile([1, P], bf16, tag="ng")
                nc.sync.dma_start(out=ng[:], in_=negrow[:])
                nc.sync.dma_start(out=T2[PADROW:PADROW + 1, :], in_=ng[:])

            # ---------------- edge phase ----------------
            _PH = int(os.environ.get("GAT_PHASES", "3"))
            with tc.tile_pool(name="eg", bufs=2) as eg, \
                 tc.tile_pool(name="ew", bufs=4) as ew, \
                 tc.tile_pool(name="eo", bufs=2) as eo, \
                 tc.tile_pool(name="eps", bufs=1, space="PSUM") as epsum:
                g1_colcur = 0
                g2_colcur = 0
                tilecur = 0
                seg_iter = 0
                sb0 = 0
                for sb, nsl in enumerate(supb_sizes):
                    if _PH < 1:
                        break
                    sl = slice(sb0, sb0 + nsl)
                    ctiles = [int(caps[sl, c].sum()) for c in range(4)]
                    stiles = sum(ctiles)
                    if stiles == 0:
                        sb0 += nsl
                        seg_iter += 4
                        continue
                    # index + dstloc loads for this superblock
                    g2it = eg.tile([P, stiles * 8], i16, tag="g2it")
                    nc.scalar.dma_start(
                        out=g2it[:], in_=g2i[:, g2_colcur:g2_colcur + stiles * 8])
                    g2_colcur += stiles * 8
                    dlt = eg.tile([P, stiles], bf16, tag="dlt")
                    nc.scalar.dma_start(
                        out=dlt[:], in_=dlp[:, tilecur:tilecur + stiles])
                    # a_dst gather buffer; filled per chunk segment below
                    g2b = eg.tile([P, stiles * P], bf16, tag="g2b")
                    if _PH < 2:
                        nc.gpsimd.memset(g2b[:], 0)

                    psum_of_slot = {}
                    flags = {}
                    for j in range(nsl):
                        g = sb0 + j
                        live = [c for c in range(4) if caps[g, c] > 0]
                        if live:
                            flags[j] = (live[0], live[-1])
                            psum_of_slot[j] = epsum.tile(
                                [P, 132], f32, space="PSUM", name=f"pslot{j}", tag=f"ps{j}")

                    srun = 0   # tile index within the supb stream
                    for c in range(4):
                        Lc = ctiles[c]
                        if Lc == 0:
                            seg_iter += 1
                            continue
                        g1it = eg.tile([P, Lc * 8], i16, tag="g1it")
                        nc.scalar.dma_start(
                            out=g1it[:], in_=g1i[:, g1_colcur:g1_colcur + Lc * 8])
                        g1_colcur += Lc * 8
                        g1b = eg.tile([P, Lc * 256], bf16, tag="g1b", bufs=3)
                        if _PH < 2:
                            nc.gpsimd.memset(g1b[:], 0)
                        _PH < 2 or nc.gpsimd.dma_gather(
                            g1b[:].rearrange("p (t c) -> p t c", t=Lc),
                            T1[c * CHUNK:(c + 1) * CHUNK, :],
                            g1it[:], Lc * P, Lc * P, 256,
                            single_packet=False)
                        _PH < 2 or nc.gpsimd.dma_gather(
                            g2b[:].rearrange("p (t c) -> p t c", t=stiles)[
                                :, srun:srun + Lc, :],
                            T2[:], g2it[:, srun * 8:(srun + Lc) * 8],
                            Lc * P, Lc * P, P, single_packet=False)
                        g1v = g1b[:].rearrange("p (t c) -> p t c", t=Lc)
                        crun = 0   # tile within this chunk segment
                        for j in range(nsl):
                            g = sb0 + j
                            K = int(caps[g, c])
                            if K == 0:
                                continue
                            po = psum_of_slot[j]
                            # logits/weights for the whole run
                            z = ew.tile([P, K * 4], f32, tag="z")
                            nc.vector.tensor_tensor(
                                out=z[:].rearrange("p (t c) -> p t c", t=K),
                                in0=g1v[:, crun:crun + K, 128:132],
                                in1=g2b[:].rearrange("p (t c) -> p t c", t=stiles)[
                                    :, srun + crun:srun + crun + K, 0:4],
                                op=AL.add)
                            e1 = ew.tile([P, K * 4], f32, tag="e1")
                            nc.scalar.activation(out=e1[:], in_=z[:], func=AF.Exp)
                            e2 = ew.tile([P, K * 4], f32, tag="e2")
                            nc.scalar.activation(out=e2[:], in_=z[:], func=AF.Exp,
                                                 scale=0.2)
                            w = ew.tile([P, K * 4], f32, tag="w")
                            nc.vector.tensor_tensor(out=w[:], in0=e1[:], in1=e2[:],
                                                    op=AL.max)
                            wv = w[:].rearrange("p (t c) -> p t c", t=K)
                            for t in range(K):
                                gt = crun + t
                                sel = ew.tile([P, P], bf16, tag="sel")
                                nc.vector.tensor_tensor(
                                    out=sel[:],
                                    in0=dlt[:, srun + gt:srun + gt + 1].to_broadcast([P, P]),
                                    in1=iot[:], op=AL.is_equal)
                                mp = ew.tile([P, 132], bf16, tag="mp")
                                nc.vector.tensor_tensor(
                                    out=mp[:, 0:128].rearrange("p (h c) -> p h c", h=4),
                                    in0=g1v[:, gt, 0:128].rearrange("p (h c) -> p h c", h=4),
                                    in1=wv[:, t, :].unsqueeze(-1).to_broadcast([P, 4, 32]),
                                    op=AL.mult)
                                nc.scalar.activation(out=mp[:, 128:132], in_=wv[:, t, :],
                                                     func=AF.Copy)
                                fc = flags[j]
                                nc.tensor.matmul(
                                    out=po[:], lhsT=sel[:], rhs=mp[:],
                                    start=(c == fc[0] and t == 0),
                                    stop=(c == fc[1] and t == K - 1))
                            crun += K
                        srun += Lc
                        seg_iter += 1
                    # flush slots
                    for j in range(nsl):
                        if j not in psum_of_slot:
                            continue
                        g = sb0 + j
                        po = psum_of_slot[j]
                        rec = ew.tile([P, 4], f32, tag="rec")
                        nc.vector.reciprocal(out=rec[:], in_=po[:, 128:132])
                        ot = eo.tile([P, P], f32, tag="ot")
                        for hh in range(4):
                            nc.vector.tensor_scalar_mul(
                                ot[:, hh * 32:(hh + 1) * 32],
                                po[:, hh * 32:(hh + 1) * 32], rec[:, hh:hh + 1])
                        nc.vector.tensor_tensor(out=ot[:], in0=ot[:], in1=bias_t[:],
                                                op=AL.add)
                        nc.sync.dma_start(out=outp[g * P:(g + 1) * P, :], in_=ot[:])
                    tilecur += stiles
                    sb0 += nsl
    nc.compile()
    if not os.environ.get("BASS_NO_WAITSPLIT"):
        _split_multi_waits(nc)
    return nc


# ---------------------------------------------------------------------------
_BUILD_CACHE = {}


def _prep_and_build(x, edge_index, W, att_src, att_dst, bias):
    cores, sched = _host_prep(np.asarray(x), np.asarray(edge_index))
    nc = _build_nc(sched)

    x = np.asarray(x, np.float32)
    xpad = np.zeros((NP_, IN_DIM), np.float32)
    xpad[:N] = x
    xT = np.ascontiguousarray(xpad.T)

    Acat = np.zeros((P, 8), np.float32)
    a_s = np.asarray(att_src, np.float32)
    a_d = np.asarray(att_dst, np.float32)
    for h in range(HEADS):
        Acat[h * CDIM:(h + 1) * CDIM, h] = a_s[h]
        Acat[h * CDIM:(h + 1) * CDIM, 4 + h] = a_d[h]
    biasr = np.tile(np.asarray(bias, np.float32)[None, :], (P, 1))
    iota = np.tile(np.arange(P, dtype=BF16)[None, :], (P, 1))
    negrow = np.full((1, P), _NEG, BF16)
    Wf = np.ascontiguousarray(np.asarray(W, np.float32))

    in_maps = []
    for k in range(NCORES):
        g1w, g2w, dl = _pack_core_arrays(cores[k], sched)
        nodes = (sched["grp"][:, k][:, None] * P + np.arange(P)[None, :]).reshape(-1)
        xsT = np.ascontiguousarray(xpad[nodes].T)
        in_maps.append({
            "xT": xT, "xsT": xsT, "W": Wf, "Acat": Acat, "biasr": biasr,
            "iota": iota, "negrow": negrow,
            "g1i": np.ascontiguousarray(g1w), "g2i": np.ascontiguousarray(g2w),
            "dloc": np.ascontiguousarray(dl),
        })
    return nc, in_maps, sched


def _assemble(results, sched):
    full = np.zeros((NP_, P), np.float32)
    grp = sched["grp"]
    for k in range(NCORES):
        o = np.asarray(results[k]["out"])        # [SHARD, 128]
        wins = grp[:, k]                         # window id per slot
        full[(wins[:, None] * P + np.arange(P)[None, :]).reshape(-1)] = o
    return full[:N]


def kernel(**inputs):
    x = inputs["x"]
    edge_index = inputs["edge_index"]
    nc, in_maps, sched = _prep_and_build(
        x, edge_index, inputs["W"], inputs["att_src"], inputs["att_dst"],
        inputs["bias"])
    res = run_bass_kernel_spmd(nc, in_maps, core_ids=list(range(NCORES)))
    return _assemble(res.results, sched)



# revision 21
# speedup vs baseline: 2.3779x; 2.3779x over previous
"""GAT layer (PyG GATConv semantics) on 8 Trainium2 NeuronCores.

Strategy (edge/graph parallel, dst-sharded, v2):
  - Append self-loops; partition destination nodes into 784 windows of 128.
  - Rank windows by edge count; window rank-group g supplies slot g of each
    of the 8 cores, so all cores share one compile-time schedule.
  - Each core builds the full node table T1c[chunk] (4 chunks of 25088 rows,
    512B rows = [h(128 bf16) | a_src(4) | junk]) via x @ [W | W@A] on the
    TensorEngine.  Rows are stored partition-major (row = (n%128)*196 + n//128)
    so table writes are 128 contiguous 3.5KB descriptors per block.
  - A small per-window a_dst table a2[d,4] (d = dst lane) lives in SBUF,
    built from the dst-shard's x columns (xsT input).
  - Edge phase (chunk-major, overlaps the remaining table builds): one
    dma_gather of T1 rows per (superblock, chunk) segment; per-edge a_dst is
    computed on-chip: SelT[d,e] = (d == dstloc_e) one-hot (int8 is_equal
    against a partition-broadcast dstloc row), then adps = SelT.T @ a2.
    w = exp(max(z, 0.2z)) with z = a_src + a_dst; the message h*w and w are
    written in-place into the gather buffer, and psum[d] += Sel.T @ [h*w | w]
    accumulates per (slot, chunk) run, added into an SBUF f32 accumulator.
  - Padding lanes carry dstloc = -1, matching no one-hot column, so they
    contribute nothing (no pad-row tricks needed).
  - Finally out[d] = acc[d]/s[d] + bias in two batched DVE ops.
"""

import math
import os

import numpy as np
import ml_dtypes

import concourse.bacc as bacc
import concourse.bass as bass
import concourse.mybir as mybir
import concourse.tile as tile
from concourse.library_config import mlp
from concourse.bass_utils import run_bass_kernel_spmd
from concourse.masks import make_identity

BF16 = ml_dtypes.bfloat16

N = 100000
E = 1600000
IN_DIM = 128
HEADS = 4
CDIM = 32
NCORES = 8
P = 128

NP_ = 100352            # N padded to 784 x-tiles of 128
NWIN = NP_ // P         # 784 global windows
SLOTS = NWIN // NCORES  # 98 slots per core
CHUNK = NP_ // 4        # 25088 rows per T1 chunk (int16-indexable)
TPC = CHUNK // P        # 196 tiles (rows-per-partition) per chunk
SHARD = SLOTS * P       # 12544 dst nodes per core
SUPB = 4                # slots per superblock (gather batching)
NBLK = 7                # table-build tiles per block (196 = 28*7)
NCHUNK = 4


# ---------------------------------------------------------------------------
# walrus workaround: this container's walrus accepts ONE sem wait per
# instruction; TileContext's tail drain accumulates many. Split extras onto
# single-wait EventSemaphore instructions.
def _split_multi_waits(nc):
    n = [0]

    def fresh():
        n[0] += 1
        return f"waitsplit-{n[0]}"

    for fn in nc.m.functions:
        for bb in fn.blocks:
            insts = list(bb.instructions)
            if not any(
                i.sync_info is not None and len(i.sync_info.on_wait) > 1
                for i in insts
            ):
                continue
            out = []
            for inst in insts:
                si = inst.sync_info
                if si is not None and len(si.on_wait) > 1:
                    waits = list(si.on_wait)
                    for w in waits[:-1]:
                        out.append(mybir.InstEventSemaphore(
                            name=fresh(), opcode="EventSemaphore",
                            engine=inst.engine,
                            sync_info=mybir.SyncInfo(on_wait=[w], on_update=[]),
                        ))
                    si.on_wait = waits[-1:]
                out.append(inst)
            bb.instructions = out


def _wrap_idx(seg):
    """dma_gather index layout: wrap in 16 partitions, replicate x8."""
    assert seg.size % 128 == 0
    return np.tile(seg.reshape(-1, 16).T, (8, 1)).astype(np.int16)


# ---------------------------------------------------------------------------
def _host_prep(x, edge_index):
    """Build the per-core schedule + data arrays. Pure indexing, no FP math."""
    src = np.concatenate([edge_index[0].astype(np.int64), np.arange(N)])
    dst = np.concatenate([edge_index[1].astype(np.int64), np.arange(N)])
    win = dst >> 7

    wcount = np.bincount(win, minlength=NWIN)
    order = np.argsort(-wcount, kind="stable")        # windows by size desc
    core_of_win = np.empty(NWIN, np.int64)
    slot_of_win = np.empty(NWIN, np.int64)
    core_of_win[order] = np.arange(NWIN) % NCORES
    slot_of_win[order] = np.arange(NWIN) // NCORES

    chunk = src // CHUNK
    wc = np.bincount(win * 4 + chunk, minlength=NWIN * 4).reshape(NWIN, 4)
    # caps[g][c]: tiles for chunk-c segment of slot g (max over the 8 cores)
    grp = order.reshape(SLOTS, NCORES)
    caps = np.ceil(wc[grp].max(axis=1) / P).astype(np.int64)   # [SLOTS, 4]

    # stream layout: chunk-major — position ordered by (chunk, supb, slot, tile)
    supb_sizes = [SUPB] * (SLOTS // SUPB) + ([SLOTS % SUPB] if SLOTS % SUPB else [])
    seg_tiles = []          # (c, sb, tiles)
    slot_seg_off = np.zeros((SLOTS, 4), np.int64)   # tile offset of (g, c) run
    tcur = 0
    for c in range(4):
        sb0 = 0
        for sb, nsl in enumerate(supb_sizes):
            for j in range(nsl):
                g = sb0 + j
                slot_seg_off[g, c] = tcur
                tcur += caps[g, c]
            seg_tiles.append((c, sb, int(caps[sb0:sb0 + nsl, c].sum())))
            sb0 += nsl
    T_tot = tcur

    # per-core arrays
    ecore = core_of_win[win]
    eslot = slot_of_win[win]
    cores = []
    for k in range(NCORES):
        m = np.nonzero(ecore == k)[0]
        es, ed, ec, eg = src[m], dst[m], chunk[m], eslot[m]
        o = np.lexsort((ed, ec, eg))
        es, ed, ec, eg = es[o], ed[o], ec[o], eg[o]
        # rank within (slot, chunk) group
        key = eg * 4 + ec
        start = np.searchsorted(key, np.arange(SLOTS * 4))
        rank = np.arange(len(es)) - start[key]
        pos = slot_seg_off[eg, ec] * P + rank
        esl = es - ec * CHUNK                         # chunk-local node id
        row = (esl & 127) * TPC + (esl >> 7)          # partition-major T1 row
        g1 = np.zeros(T_tot * P, np.int16)            # pad: row 0 (masked)
        dl = np.full(T_tot * P, -1, np.int8)          # pad: dstloc -1
        g1[pos] = row.astype(np.int16)
        dl[pos] = (ed & 127).astype(np.int8)
        cores.append({"g1": g1, "dl": dl})

    sched = {
        "caps": caps, "supb_sizes": supb_sizes, "seg_tiles": seg_tiles,
        "T_tot": T_tot, "order": order, "grp": grp,
        "core_of_win": core_of_win, "slot_of_win": slot_of_win,
    }
    return cores, sched


def _pack_core_arrays(core, sched):
    """Wrap index streams per gather instruction; dstloc per tile column."""
    T_tot = sched["T_tot"]
    g1_parts = []
    t0 = 0
    for (c, sb, tiles) in sched["seg_tiles"]:
        if tiles:
            g1_parts.append(_wrap_idx(core["g1"][t0 * P:(t0 + tiles) * P]))
        t0 += tiles
    g1w = np.concatenate(g1_parts, axis=1) if g1_parts else np.zeros((128, 0), np.int16)
    dlt = core["dl"].reshape(T_tot, P).T.astype(BF16)   # [128, T_tot]
    return g1w, dlt


# ---------------------------------------------------------------------------
def _build_nc(sched):
    caps = sched["caps"]
    supb_sizes = sched["supb_sizes"]
    T_tot = sched["T_tot"]
    AF = mybir.ActivationFunctionType
    AL = mybir.AluOpType
    f32, bf16 = mybir.dt.float32, mybir.dt.bfloat16
    i16, i8 = mybir.dt.int16, mybir.dt.int8

    g1cols = sum(t * 8 for (_, _, t) in sched["seg_tiles"])
    LCMAX = max(t for (_, _, t) in sched["seg_tiles"])
    # first chunk with edges, per slot (acc copy-vs-add selector)
    first_c = [int(np.nonzero(caps[g])[0][0]) for g in range(SLOTS)]

    nc = bacc.Bacc("TRN2")
    xT = nc.declare_dram_parameter("xT", [P, NP_], bf16, isOutput=False)
    xsT = nc.declare_dram_parameter("xsT", [P, SHARD], bf16, isOutput=False)
    Wp = nc.declare_dram_parameter("W", [P, P], bf16, isOutput=False)
    Acat = nc.declare_dram_parameter("Acat", [P, 8], bf16, isOutput=False)
    biasr = nc.declare_dram_parameter("biasr", [P, P], f32, isOutput=False)
    iotbp = nc.declare_dram_parameter("iotB", [P, P * LCMAX], bf16, isOutput=False)
    g1i = nc.declare_dram_parameter("g1i", [P, max(g1cols, 8)], i16, isOutput=False)
    dlp = nc.declare_dram_parameter("dlp", [P, max(T_tot, 1)], bf16, isOutput=False)
    outp = nc.declare_dram_parameter("out", [SHARD, P], f32, isOutput=True)

    T1c = [nc.dram_tensor(f"t1c{c}", [CHUNK, 256], bf16) for c in range(NCHUNK)]

    nc.gpsimd.load_library(mlp)

    _PH = int(os.environ.get("GAT_PHASES", "3"))

    with tile.TileContext(nc) as tc:
        with tc.tile_pool(name="const", bufs=1) as cpool:
            ident = cpool.tile([P, P], f32)
            make_identity(nc, ident[:])
            identb = cpool.tile([P, P], bf16)
            make_identity(nc, identb[:])
            iotB = cpool.tile([P, P * LCMAX], bf16)
            nc.sync.dma_start(out=iotB[:], in_=iotbp[:])
            bias_t = cpool.tile([P, P], f32)
            nc.sync.dma_start(out=bias_t[:], in_=biasr[:])
            wprime = cpool.tile([P, 136], bf16)
            nc.sync.dma_start(out=wprime[:, 0:128], in_=Wp[:])
            acat_t = cpool.tile([P, 8], bf16)
            nc.sync.dma_start(out=acat_t[:], in_=Acat[:])
            dlt = cpool.tile([P, max(T_tot, 1)], bf16)
            nc.scalar.dma_start(out=dlt[:], in_=dlp[:])
            acc = cpool.tile([P, SLOTS * 132], f32)
            a2sb = cpool.tile([P, SLOTS * 4], bf16)

            with tc.tile_pool(name="tb", bufs=2) as tb, \
                 tc.tile_pool(name="tbp", bufs=2, space="PSUM") as tbp, \
                 tc.tile_pool(name="eg", bufs=4) as eg, \
                 tc.tile_pool(name="ew", bufs=4) as ew, \
                 tc.tile_pool(name="epo", bufs=2, space="PSUM") as epo, \
                 tc.tile_pool(name="ead", bufs=2, space="PSUM") as ead, \
                 tc.tile_pool(name="etp", bufs=2, space="PSUM") as etp:
                # ---- W' cols 128:136 = W @ Acat (contract over out-features)
                wtp = tbp.tile([P, 136], bf16, space="PSUM", tag="ps")
                nc.tensor.transpose(out=wtp[:, 0:128], in_=wprime[:, 0:128],
                                    identity=identb[:])
                wT = tb.tile([P, P], bf16, tag="wT")
                nc.vector.tensor_copy(out=wT[:], in_=wtp[:, 0:128])
                wap = tbp.tile([P, 136], f32, space="PSUM", tag="ps")
                nc.tensor.matmul(out=wap[:, 0:8], lhsT=wT[:], rhs=acat_t[:],
                                 start=True, stop=True)
                nc.vector.tensor_copy(out=wprime[:, 128:136], in_=wap[:, 0:8])

                # ---- a2 table: a_dst for the shard's windows, [d, 4] each
                for b in range(SLOTS // NBLK):
                    xs = tb.tile([P, NBLK * P], bf16, tag="xt")
                    nc.sync.dma_start(
                        out=xs[:], in_=xsT[:, b * NBLK * P:(b + 1) * NBLK * P])
                    a2p = tbp.tile([P, 136], f32, space="PSUM", tag="ps")
                    for j in range(NBLK):
                        nc.tensor.matmul(
                            out=a2p[:, j * 4:(j + 1) * 4],
                            lhsT=xs[:, j * P:(j + 1) * P],
                            rhs=wprime[:, 132:136], start=True, stop=True)
                    g0 = b * NBLK
                    nc.vector.tensor_copy(
                        out=a2sb[:, g0 * 4:(g0 + NBLK) * 4], in_=a2p[:, 0:NBLK * 4])

                # ---- node table build, chunk-major
                for c in range(NCHUNK):
                    for b in range(TPC // NBLK):
                        xt = tb.tile([P, NBLK * P], bf16, tag="xt")
                        off = c * CHUNK + b * NBLK * P
                        nc.sync.dma_start(
                            out=xt[:], in_=xT[:, off:off + NBLK * P])
                        st = tb.tile([P, NBLK * 256], bf16, tag="st")
                        for t in range(NBLK):
                            ps = tbp.tile([P, 136], f32, space="PSUM", tag="ps")
                            nc.tensor.matmul(
                                out=ps[:], lhsT=xt[:, t * P:(t + 1) * P],
                                rhs=wprime[:], start=True, stop=True)
                            if t % 2 == 0:
                                nc.vector.tensor_copy(
                                    out=st[:, t * 256:t * 256 + 132],
                                    in_=ps[:, 0:132])
                            else:
                                nc.scalar.activation(
                                    out=st[:, t * 256:t * 256 + 132],
                                    in_=ps[:, 0:132], func=AF.Copy)
                        # fill junk cols so full 512B rows can be written
                        stv = st[:].rearrange("p (t c) -> p t c", t=NBLK)
                        nc.vector.tensor_copy(
                            out=stv[:, :, 132:256], in_=stv[:, :, 0:124])
                        nc.sync.dma_start(
                            out=T1c[c][:].rearrange("(p t) c -> p t c", p=P)[
                                :, b * NBLK:(b + 1) * NBLK, :],
                            in_=stv)

                # ---- edge phase, chunk-major, software-pipelined emission
                segs = []
                g1col = 0
                tilecur = 0
                for (c, sb, Lc) in sched["seg_tiles"]:
                    if Lc:
                        sb0 = sum(supb_sizes[:sb])
                        runs = []
                        t0 = 0
                        for j in range(supb_sizes[sb]):
                            g = sb0 + j
                            K = int(caps[g, c])
                            if K:
                                runs.append((g, t0, K))
                                t0 += K
                        assert t0 == Lc
                        segs.append((c, Lc, tilecur, g1col, runs))
                    g1col += Lc * 8
                    tilecur += Lc
                if _PH < 1:
                    segs = []
                state = {}

                def stage_a(i):
                    """gather + Sel one-hot + PE transposes of Sel."""
                    c, Lc, tc_, gc_, runs = segs[i]
                    g1it = eg.tile([P, Lc * 8], i16, tag="g1it")
                    nc.scalar.dma_start(
                        out=g1it[:], in_=g1i[:, gc_:gc_ + Lc * 8])
                    g1b = eg.tile([P, Lc * 256], bf16, tag="g1b")
                    g1v = g1b[:].rearrange("p (t c) -> p t c", t=Lc)
                    if _PH < 2:
                        nc.gpsimd.memset(g1b[:], 0)
                    else:
                        nc.gpsimd.dma_gather(
                            g1v, T1c[c][:], g1it[:], Lc * P, Lc * P, 256,
                            single_packet=False)
                    # Sel one-hot, d-major: sel[e, (d, t)] = (dl[e,t] == d)
                    sel = ew.tile([P, P * Lc], bf16, tag="sel")
                    selv = sel[:].rearrange("p (d t) -> p d t", d=P)
                    nc.vector.tensor_tensor(
                        out=selv, op=AL.is_equal,
                        in0=dlt[:, tc_:tc_ + Lc].unsqueeze(1)
                            .to_broadcast([P, P, Lc]),
                        in1=iotB[:].rearrange("p (d t) -> p d t", d=P)[
                            :, :, 0:Lc])
                    tps = []
                    for q0 in range(0, Lc, 8):
                        qn = min(8, Lc - q0)
                        tp = etp.tile([P, 1024], bf16, space="PSUM", tag="tp")
                        for q in range(qn):
                            nc.tensor.transpose(
                                out=tp[:, q * P:(q + 1) * P],
                                in_=selv[:, :, q0 + q], identity=identb[:])
                        tps.append((q0, qn, tp))
                    state[i] = [g1v, selv, tps, None, None]

                def stage_b(i):
                    """evict SelT psums (ACT) + per-edge a_dst matmuls."""
                    c, Lc, tc_, gc_, runs = segs[i]
                    g1v, selv, tps, _, _ = state[i]
                    selT = ew.tile([P, Lc * P], bf16, tag="selT")
                    for (q0, qn, tp) in tps:
                        nc.scalar.activation(
                            out=selT[:, q0 * P:(q0 + qn) * P],
                            in_=tp[:, 0:qn * P], func=AF.Copy)
                    adps = ead.tile([P, Lc * 4], f32, space="PSUM", tag="adps")
                    for (g, t0, K) in runs:
                        for t in range(t0, t0 + K):
                            nc.tensor.matmul(
                                out=adps[:, t * 4:(t + 1) * 4],
                                lhsT=selT[:, t * P:(t + 1) * P],
                                rhs=a2sb[:, g * 4:(g + 1) * 4],
                                start=True, stop=True)
                    state[i][2] = None
                    state[i][3] = adps

                def stage_c1(i):
                    """z = a_src + a_dst ; w = exp(max(z, .2z))."""
                    c, Lc, tc_, gc_, runs = segs[i]
                    g1v, selv, _, adps, _ = state[i]
                    z = ew.tile([P, Lc * 4], f32, tag="z")
                    nc.vector.tensor_tensor(
                        out=z[:].rearrange("p (t h) -> p t h", t=Lc),
                        in0=g1v[:, :, 128:132],
                        in1=adps[:].rearrange("p (t h) -> p t h", t=Lc),
                        op=AL.add)
                    lr = ew.tile([P, Lc * 4], f32, tag="lr")
                    nc.vector.scalar_tensor_tensor(
                        out=lr[:], in0=z[:], scalar=0.2, in1=z[:],
                        op0=AL.mult, op1=AL.max)
                    nc.scalar.activation(
                        out=g1v[:, :, 128:132],
                        in_=lr[:].rearrange("p (t h) -> p t h", t=Lc),
                        func=AF.Exp)

                def stage_c2(i):
                    """messages in place, aggregate per run, accumulate."""
                    c, Lc, tc_, gc_, runs = segs[i]
                    g1v, selv, _, adps, _ = state[i]
                    nc.vector.tensor_tensor(
                        out=g1v[:, :, 0:128].rearrange(
                            "p t (cc h) -> p t cc h", h=4),
                        op=AL.mult,
                        in0=g1v[:, :, 0:128].rearrange(
                            "p t (cc h) -> p t cc h", h=4),
                        in1=g1v[:, :, 128:132].unsqueeze(2)
                            .to_broadcast([P, Lc, 32, 4]))
                    pos = []
                    for (g, t0, K) in runs:
                        po = epo.tile([P, 132], f32, space="PSUM", tag="po")
                        for q, t in enumerate(range(t0, t0 + K)):
                            nc.tensor.matmul(
                                out=po[:], lhsT=selv[:, :, t],
                                rhs=g1v[:, t, 0:132],
                                start=(q == 0), stop=(q == K - 1))
                        pos.append((g, po))
                    state[i][4] = pos

                def stage_d(i):
                    """accumulate psums into SBUF acc."""
                    c = segs[i][0]
                    for (g, po) in state[i][4]:
                        aslice = acc[:, g * 132:(g + 1) * 132]
                        if c == first_c[g]:
                            nc.vector.tensor_copy(out=aslice, in_=po[:])
                        else:
                            nc.vector.tensor_tensor(
                                out=aslice, in0=aslice, in1=po[:], op=AL.add)
                    del state[i]

                def emit_out(runs):
                    """normalize + bias + store finished slots."""
                    gs = [g for (g, _, _) in runs]
                    g0, gn = gs[0], len(gs)
                    assert gs == list(range(g0, g0 + gn))
                    accw = acc[:, g0 * 132:(g0 + gn) * 132].rearrange(
                        "p (j m) -> p j m", j=gn)
                    rec = ew.tile([P, gn * 4], f32, tag="rec")
                    nc.vector.reciprocal(
                        out=rec[:].rearrange("p (j h) -> p j h", j=gn),
                        in_=accw[:, :, 128:132])
                    ot = ew.tile([P, gn * P], f32, tag="ot")
                    otv = ot[:].rearrange("p (j c) -> p j c", j=gn)
                    nc.vector.tensor_tensor(
                        out=otv.rearrange("p j (cc h) -> p j cc h", h=4),
                        in0=accw[:, :, 0:128].rearrange(
                            "p j (cc h) -> p j cc h", h=4),
                        in1=rec[:].rearrange("p (j h) -> p j h", j=gn)
                            .unsqueeze(2).to_broadcast([P, gn, CDIM, 4]),
                        op=AL.mult)
                    nc.vector.tensor_tensor(
                        out=otv, in0=otv,
                        in1=bias_t[:].unsqueeze(1).to_broadcast([P, gn, P]),
                        op=AL.add)
                    nc.sync.dma_start(
                        out=outp[g0 * P:(g0 + gn) * P, :].rearrange(
                            "(j p) c -> p j c", p=P),
                        in_=otv)

                ns = len(segs)
                if ns:
                    stage_a(0)
                    stage_b(0)
                    if ns > 1:
                        stage_a(1)
                for i in range(ns):
                    stage_c1(i)
                    stage_c2(i)
                    if i + 1 < ns:
                        stage_b(i + 1)
                    if i + 2 < ns:
                        stage_a(i + 2)
                    stage_d(i)
                    if segs[i][0] == NCHUNK - 1:
                        emit_out(segs[i][4])

    nc.compile()
    if not os.environ.get("BASS_NO_WAITSPLIT"):
        _split_multi_waits(nc)
    return nc


# ---------------------------------------------------------------------------
def _prep_and_build(x, edge_index, W, att_src, att_dst, bias):
    cores, sched = _host_prep(np.asarray(x), np.asarray(edge_index))
    nc = _build_nc(sched)

    x = np.asarray(x, np.float32)
    xpad = np.zeros((NP_, IN_DIM), BF16)
    xpad[:N] = x.astype(BF16)
    xT = np.ascontiguousarray(xpad.T)

    # out-features reordered c-major: new col c*4+h = old col h*32+c
    perm = np.array([h * CDIM + c for c in range(CDIM) for h in range(HEADS)])
    a_s = np.asarray(att_src, np.float32)
    a_d = np.asarray(att_dst, np.float32)
    Acat = np.zeros((P, 8), BF16)
    for h in range(HEADS):
        for c in range(CDIM):
            Acat[c * HEADS + h, h] = a_s[h, c]
            Acat[c * HEADS + h, 4 + h] = a_d[h, c]
    biasr = np.tile(np.asarray(bias, np.float32)[perm][None, :], (P, 1))
    LCMAX = max(t for (_, _, t) in sched["seg_tiles"])
    iotB = np.tile(
        np.repeat(np.arange(P), LCMAX).astype(BF16)[None, :], (P, 1))
    Wf = np.ascontiguousarray(np.asarray(W, np.float32)[:, perm].astype(BF16))

    in_maps = []
    for k in range(NCORES):
        g1w, dlt = _pack_core_arrays(cores[k], sched)
        nodes = (sched["grp"][:, k][:, None] * P + np.arange(P)[None, :]).reshape(-1)
        xsT = np.ascontiguousarray(xpad[nodes].T)
        in_maps.append({
            "xT": xT, "xsT": xsT, "W": Wf, "Acat": Acat, "biasr": biasr,
            "iotB": iotB,
            "g1i": np.ascontiguousarray(g1w),
            "dlp": np.ascontiguousarray(dlt),
        })
    return nc, in_maps, sched


def _assemble(results, sched):
    perm = np.array([h * CDIM + c for c in range(CDIM) for h in range(HEADS)])
    inv = np.empty(P, np.int64)
    inv[perm] = np.arange(P)
    full = np.zeros((NP_, P), np.float32)
    grp = sched["grp"]
    for k in range(NCORES):
        o = np.asarray(results[k]["out"])[:, inv]   # [SHARD, 128], (h,c) order
        wins = grp[:, k]                            # window id per slot
        full[(wins[:, None] * P + np.arange(P)[None, :]).reshape(-1)] = o
    return full[:N]


def kernel(**inputs):
    x = inputs["x"]
    edge_index = inputs["edge_index"]
    nc, in_maps, sched = _prep_and_build(
        x, edge_index, inputs["W"], inputs["att_src"], inputs["att_dst"],
        inputs["bias"])
    res = run_bass_kernel_spmd(nc, in_maps, core_ids=list(range(NCORES)))
    return _assemble(res.results, sched)
